# revision 3
# baseline (speedup 1.0000x reference)
"""Distributed AttentionLayer kernel for one TRN2 chip (8 NeuronCores).

Reference computation (note the unusual softmax over the QUERY axis):
    Q = Xq @ Wq.T + bq                      [B, L, 128]
    K = Xk @ Wk.T + bk
    V = Xv @ Wv.T + bv
    S = softmax(Q @ K.T / sqrt(128), axis=q)    (normalized over queries)
    H = S @ V                               [B, L, 128]

Sharding: 8 cores = 4 batches x 2 key-chunks; exact partial-H per core
(zero collectives), host adds the two k-chunk partials per batch.

All-bf16 value path (fp8 rejected: per-element quantization noise of any
operand in the E/V/H chain lands on H at full strength -- for iid data
noise and signal both grow as sqrt(N), nothing washes out -- and fp8's
~3% rms blows the 2e-2 gate).

v4: the schedule is rebuilt around the measured DMA arrival train.  The
input stream (17.4MB on one HWDGE queue) runs at a steady ~415GB/s and
is the startup pacer; v3 wasted 12us by (a) shipping xv0 before
xq0/xk0, (b) wave0 needing BOTH xk0 and xk1, (c) 36 V matmuls parked in
the PE's in-order FIFO ahead of the first score matmuls.  v4 ships
tensors in exact first-use order, opens with a minimal 3.9MB prefix
(consts_q + xq0 + xq1 + consts_kv + xk0 -> first exp ~18us vs 30.3us),
and weaves the V projections into the sweep where xv arrives.  ET/hsum
SBUF slots are paired with input tiles that are provably dead >=1.2us
before first write.  PE warmup matmuls (full 128-partition stationary)
open the HAM clock gate during the DMA lead-in.

Engine budget per core: ACT 64 exps ~72us + 16 colsum accums (the
qp0 partials ride the exp ACTIVATE's accumulator); DVE carries qp1-3
colsum reduces + evacuations (~75us); GPSIMD takes the Vs scaling;
PE ~107us busy incl. overheads.
"""

import math

import numpy as np
import ml_dtypes

B, L, DM, DH = 4, 4096, 1024, 128
NCORES = 8
KCH = L // 2            # 2048 keys per core
QCS = 512               # matmul moving-dim chunk (one PSUM bank of f32)
NQC = L // QCS          # 8
PCS = 1024              # 2-bank PSUM tile width (pairs of QCS chunks)
NPC = L // PCS          # 4 query pair-chunks
NKT = KCH // 128        # 16 key tiles per core
NDT = DM // 128         # 8 d_model tiles
NKC = KCH // QCS        # 4 key 512-chunks for the K/V loads
SCALE = 1.0 / math.sqrt(DH)

_CACHE = {}


def _build():
    import concourse.tile as tile
    from concourse import bacc, mybir

    f32 = mybir.dt.float32
    bf16 = mybir.dt.bfloat16
    AX = mybir.AxisListType
    ALU = mybir.AluOpType
    ACT = mybir.ActivationFunctionType

    nc = bacc.Bacc("TRN2", target_bir_lowering=False, debug=False,
                   num_devices=NCORES)

    # Host-side layouts:
    #   x*_t: [blk, p, dt, c]  with d = dt*128+p and l = blk*512+c
    #   consts_q:  [p, 10, 128] bf16: rows 0-7 wq (W.T tiled [p, dt, o]),
    #              row 8 biases (col0 bq, col1 bk), row 9 bv replicated
    #   consts_kv: [p, 16, 128] bf16: rows 0-7 wk, rows 8-15 wv
    xq_d = nc.dram_tensor("xq_t", [NQC, 128, NDT, QCS], bf16, kind="ExternalInput")
    xk_d = nc.dram_tensor("xk_t", [NKC, 128, NDT, QCS], bf16, kind="ExternalInput")
    xv_d = nc.dram_tensor("xv_t", [NKC, 128, NDT, QCS], bf16, kind="ExternalInput")
    cq_d = nc.dram_tensor("consts_q", [128, 10, 128], bf16, kind="ExternalInput")
    ck_d = nc.dram_tensor("consts_kv", [128, 16, 128], bf16, kind="ExternalInput")
    out_d = nc.dram_tensor("out", [DH, L], f32, kind="ExternalOutput")

    with tile.TileContext(nc) as tc:
        with tc.tile_pool(name="const", bufs=1) as cpool, \
             tc.tile_pool(name="persist", bufs=1) as ppool, \
             tc.tile_pool(name="psmm", bufs=3, space="PSUM") as psmm, \
             tc.tile_pool(name="psaux", bufs=2, space="PSUM") as psaux:

            # ---------- constants ----------
            cq_sb = cpool.tile([128, 10, 128], bf16, name="cq_sb", tag="cq")
            ck_sb = cpool.tile([128, 16, 128], bf16, name="ck_sb", tag="ck")
            warm_st = cpool.tile([128, DH], bf16, name="warm_st", tag="wst")
            warm_mv = cpool.tile([128, QCS], bf16, name="warm_mv", tag="wmv")
            ones_sb = cpool.tile([1, DH], bf16, name="ones_sb", tag="ones")

            wq_sb = cq_sb[:, 0:NDT, :]
            bv_sb = cq_sb[0:1, 9, :]
            wk_sb = ck_sb[:, 0:NDT, :]
            wv_sb = ck_sb[:, NDT:2 * NDT, :]

            # consts_q leads the single sync HWDGE queue; everything ships
            # in exact first-use order (single queue = in-order delivery).
            nc.sync.dma_start(out=cq_sb[:], in_=cq_d[:])
            nc.vector.memset(warm_st[:], 0.125)
            nc.vector.memset(warm_mv[:], 0.125)
            nc.vector.memset(ones_sb[:], 1.0)

            # Per-v_group gated copies of bv: bv2[g] = 0*et_gate + bv.  The
            # artificial data dep on a mid-stream ET slice stops the legacy
            # CoreSim scheduler (whose DMA model ignores queue serialization)
            # from hoisting the V matmuls to the front of the PE FIFO where
            # they head-of-line block on xv0 for ~25us (measured in v4;
            # bass_priority is ignored by this scheduler).
            bv2_sb = cpool.tile([1, 4, DH], bf16, name="bv2_sb", tag="bv2")

            # tensor_scalar_add wants f32 scalars; up-convert the bf16
            # biases once (tiny DVE copy).
            bqk_sb = cpool.tile([128, 2], f32, name="bqk_sb", tag="bqk")
            nc.vector.tensor_copy(out=bqk_sb[:], in_=cq_sb[:, 8, 0:2])
            bq_sb = bqk_sb[:, 0:1]
            bk_sb = bqk_sb[:, 1:2]

            # ---------- persistent activations ----------
            qt_sb = ppool.tile([128, L], bf16, name="qt_sb", tag="qt")      # Q^T [o, q]
            kt_sb = ppool.tile([128, KCH], bf16, name="kt_sb", tag="kt")    # K^T [o, k]
            v_sb = ppool.tile([128, NKT, DH], bf16, name="v_sb", tag="v")   # V  [k, kt, v]
            vs_sb = ppool.tile([128, NKT, DH], bf16, name="vs_sb", tag="vs")
            cs_parts = ppool.tile([128, NKT, 5], f32, name="cs_parts", tag="csp")
            cs_sum = ppool.tile([128, NKT], f32, name="cs_sum", tag="css")
            cs_rec = ppool.tile([128, NKT], f32, name="cs_rec", tag="csr")
            # slot 4 is only written by the kt0-3 half-cells; zero the rest
            nc.vector.memset(cs_parts[:], 0.0)

            # X, ET and hsum tiles share one 20-slot 8KB rotation; the
            # allocation order pairs each late tile with an input tile
            # that dies >=1.2us before the late tile's first write
            # (arrival/death schedule in the header comment).
            with tc.tile_pool(name="xe", bufs=20) as xpool:
                xq_sbs = [None] * NQC
                xk_sbs = [None] * NKC
                xv_sbs = [None] * NKC
                alloc_seq = [("q", 0), ("q", 1), ("k", 0), ("k", 1),
                             ("q", 2), ("q", 3), ("k", 2),
                             ("q", 4), ("q", 5), ("q", 6), ("q", 7),
                             ("k", 3),
                             ("v", 0), ("v", 1), ("v", 2), ("v", 3)]
                for kind, j in alloc_seq:
                    t = xpool.tile([128, NDT, QCS], bf16,
                                   name=f"x{kind}_sb{j}", tag="xe")
                    if kind == "q":
                        xq_sbs[j] = t
                    elif kind == "k":
                        xk_sbs[j] = t
                    else:
                        xv_sbs[j] = t
                et_ts = [xpool.tile([128, L], bf16, name=f"et_sb{kt}",
                                    tag="xe") for kt in range(NKT)]
                hsum_ts = [xpool.tile([128, L // 2], f32, name=f"hsum{i}",
                                      tag="xe") for i in range(2)]

                # DMA ship order == first-use order.
                nc.sync.dma_start(out=xq_sbs[0][:], in_=xq_d[0])
                nc.sync.dma_start(out=ck_sb[:], in_=ck_d[:])
                nc.sync.dma_start(out=xk_sbs[0][:], in_=xk_d[0])
                nc.sync.dma_start(out=xq_sbs[1][:], in_=xq_d[1])
                for kind, j in [("k", 1), ("q", 2), ("q", 3),
                                ("k", 2), ("q", 4), ("q", 5), ("q", 6),
                                ("q", 7), ("k", 3),
                                ("v", 0), ("v", 1), ("v", 2), ("v", 3)]:
                    if kind == "q":
                        nc.sync.dma_start(out=xq_sbs[j][:], in_=xq_d[j])
                    elif kind == "k":
                        nc.sync.dma_start(out=xk_sbs[j][:], in_=xk_d[j])
                    else:
                        nc.sync.dma_start(out=xv_sbs[j][:], in_=xv_d[j])

                # ---------- PE warmup: open the HAM clock gate ----------
                # Full 128-partition matmuls on memset data (1-partition
                # rank-1 warmups in v3 did NOT trip the activity monitor).
                warm_ps = psaux.tile([128, QCS], f32, name="warm_ps",
                                     tag="aux")
                for i in range(12):
                    nc.tensor.matmul(out=warm_ps[:], lhsT=warm_st[:],
                                     rhs=warm_mv[:], start=True, stop=True)

                def qt_half(qc):
                    """One 512-wide Q-projection chunk (startup only: lets
                    the first exps fire after xq0 alone, ~6us earlier)."""
                    qh_ps = psmm.tile([128, QCS], f32, name=f"qh_ps{qc}",
                                      tag="mm")
                    for dt in range(NDT):
                        nc.tensor.matmul(
                            out=qh_ps[:],
                            lhsT=wq_sb[:, dt, :],
                            rhs=xq_sbs[qc][:, dt, :],
                            start=(dt == 0), stop=(dt == NDT - 1))
                    nc.vector.tensor_scalar_add(
                        out=qt_sb[:, qc * QCS:(qc + 1) * QCS], in0=qh_ps[:],
                        scalar1=bq_sb)

                def st_half(kt, h):
                    """512-wide score+exp half-cell for the first wave; the
                    colsum partial goes to slot 0 (h=0) or slot 4 (h=1)."""
                    sh_ps = psmm.tile([128, QCS], f32,
                                      name=f"sh_ps_{kt}_{h}", tag="mm")
                    nc.tensor.matmul(
                        out=sh_ps[:],
                        lhsT=kt_sb[:, kt * 128:(kt + 1) * 128],
                        rhs=qt_sb[:, h * QCS:(h + 1) * QCS],
                        start=True, stop=True)
                    slot = 0 if h == 0 else 4
                    nc.scalar.activation(
                        out=et_ts[kt][:, h * QCS:(h + 1) * QCS],
                        in_=sh_ps[:], func=ACT.Exp, scale=SCALE,
                        accum_out=cs_parts[:, kt, slot:slot + 1])

                def qt_proj(qp):
                    qt_ps = psmm.tile([128, PCS], f32, name=f"qt_ps{qp}", tag="mm")
                    for half in range(2):
                        qc = 2 * qp + half
                        for dt in range(NDT):
                            nc.tensor.matmul(
                                out=qt_ps[:, half * QCS:(half + 1) * QCS],
                                lhsT=wq_sb[:, dt, :],
                                rhs=xq_sbs[qc][:, dt, :],
                                start=(dt == 0), stop=(dt == NDT - 1))
                    nc.vector.tensor_scalar_add(
                        out=qt_sb[:, qp * PCS:(qp + 1) * PCS], in0=qt_ps[:],
                        scalar1=bq_sb)

                def kt_proj(kc):
                    kt_ps = psmm.tile([128, QCS], f32, name=f"kt_ps{kc}", tag="mm")
                    for dt in range(NDT):
                        nc.tensor.matmul(
                            out=kt_ps[:],
                            lhsT=wk_sb[:, dt, :],
                            rhs=xk_sbs[kc][:, dt, :],
                            start=(dt == 0), stop=(dt == NDT - 1))
                    nc.vector.tensor_scalar_add(
                        out=kt_sb[:, kc * QCS:(kc + 1) * QCS], in0=kt_ps[:],
                        scalar1=bk_sb)

                def st_cell(kt, qp):
                    """Scores + exp for one (kt, qp) cell; colsum partial on
                    the ACT accumulator for qp0, DVE reduce for qp1-3."""
                    st_ps = psmm.tile([128, PCS], f32,
                                      name=f"st_ps_{kt}_{qp}", tag="mm")
                    for half in range(2):
                        qc = 2 * qp + half
                        nc.tensor.matmul(
                            out=st_ps[:, half * QCS:(half + 1) * QCS],
                            lhsT=kt_sb[:, kt * 128:(kt + 1) * 128],
                            rhs=qt_sb[:, qc * QCS:(qc + 1) * QCS],
                            start=True, stop=True)
                    et_slice = et_ts[kt][:, qp * PCS:(qp + 1) * PCS]
                    if qp == 0:
                        nc.scalar.activation(
                            out=et_slice, in_=st_ps[:], func=ACT.Exp,
                            scale=SCALE, accum_out=cs_parts[:, kt, qp:qp + 1])
                    else:
                        nc.scalar.activation(
                            out=et_slice, in_=st_ps[:], func=ACT.Exp,
                            scale=SCALE)
                        nc.vector.tensor_reduce(
                            out=cs_parts[:, kt, qp:qp + 1], in_=et_slice,
                            axis=AX.X, op=ALU.add)

                def finish_kt(kt):
                    """Total colsum -> reciprocal -> scaled V for one k-tile."""
                    nc.vector.tensor_reduce(
                        out=cs_sum[:, kt:kt + 1], in_=cs_parts[:, kt, :],
                        axis=AX.X, op=ALU.add)
                    nc.vector.reciprocal(out=cs_rec[:, kt:kt + 1],
                                         in_=cs_sum[:, kt:kt + 1])
                    nc.vector.tensor_scalar_mul(
                        out=vs_sb[:, kt, :], in0=v_sb[:, kt, :],
                        scalar1=cs_rec[:, kt:kt + 1])

                def v_group(g, gate):
                    """V[k,v] for k-tiles 4g..4g+3 (one xv block): bias
                    rank-1 + 8 d-tiles each; one PSUM bank, one evac.  The
                    bias matmul reads bv2[g] whose producer depends on
                    `gate`, anchoring the whole PSUM accumulation group at
                    the right spot in the PE FIFO."""
                    nc.vector.scalar_tensor_tensor(
                        out=bv2_sb[0:1, g, :], in0=gate, scalar=0.0,
                        in1=bv_sb, op0=ALU.mult, op1=ALU.add)
                    v_ps = psaux.tile([128, 4, DH], f32, name=f"v_ps{g}",
                                      tag="aux")
                    for i in range(4):
                        nc.tensor.matmul(out=v_ps[:, i, :], lhsT=ones_sb[:],
                                         rhs=bv2_sb[0:1, g, :],
                                         start=True, stop=False)
                        for dt in range(NDT):
                            nc.tensor.matmul(
                                out=v_ps[:, i, :],
                                lhsT=xv_sbs[g][:, dt, i * 128:(i + 1) * 128],
                                rhs=wv_sb[:, dt, :],
                                start=False, stop=(dt == NDT - 1))
                    nc.vector.tensor_copy(out=v_sb[:, 4 * g:4 * g + 4, :],
                                          in_=v_ps[:])

                def bh_group(qc, kts, emit_out):
                    """H^T partial over `kts` for one q-column; first half
                    parks in hsum (f32 SBUF), second half adds it back."""
                    ht_ps = psaux.tile([128, QCS], f32,
                                       name=f"ht_ps_{qc}_{kts[0]}", tag="aux")
                    for i, kt in enumerate(kts):
                        nc.tensor.matmul(
                            out=ht_ps[:],
                            lhsT=vs_sb[:, kt, :],
                            rhs=et_ts[kt][:, qc * QCS:(qc + 1) * QCS],
                            start=(i == 0), stop=(i == len(kts) - 1))
                    hslice = hsum_ts[qc // 4][:, (qc % 4) * QCS:
                                              (qc % 4 + 1) * QCS]
                    if not emit_out:
                        nc.vector.tensor_copy(out=hslice, in_=ht_ps[:])
                        return None
                    return ht_ps, hslice

                # ---------- schedule: exact DMA-arrival order ----------
                qt_half(0)
                kt_proj(0)
                for kt in range(0, 4):
                    st_half(kt, 0)
                qt_half(1)
                for kt in range(0, 4):
                    st_half(kt, 1)
                kt_proj(1)
                for kt in range(4, 8):
                    st_cell(kt, 0)
                qt_proj(1)
                for kt in range(0, 8):
                    st_cell(kt, 1)
                kt_proj(2)
                st_cell(8, 0)
                st_cell(9, 0)
                qt_proj(2)
                st_cell(10, 0)
                st_cell(11, 0)
                qt_proj(3)
                kt_proj(3)
                for kt in range(12, 16):
                    st_cell(kt, 0)
                for kt in range(8, 16):
                    st_cell(kt, 1)

                # kt-major sweep: each k-tile's colsum closes right after
                # its (qp2, qp3) cells; V projections woven where xv lands;
                # kt0-7 H partials woven from kt>=8.
                # Gates: ET slices produced around each xv block's real
                # arrival time (st(12,0) ~46us, st(9,1) ~48us, ...).
                VG_AT = {0: (0, et_ts[12][0:1, 0:DH]),
                         1: (1, et_ts[9][0:1, PCS:PCS + DH]),
                         3: (2, et_ts[11][0:1, PCS:PCS + DH]),
                         5: (3, et_ts[13][0:1, PCS:PCS + DH])}
                for kt in range(NKT):
                    st_cell(kt, 2)
                    if kt in VG_AT:
                        g, gate = VG_AT[kt]
                        v_group(g, gate)
                    st_cell(kt, 3)
                    finish_kt(kt)
                    if kt >= 8:
                        bh_group(kt - 8, list(range(8)), emit_out=False)

                # ---------- H^T second half (kt8-15) + combine ----------
                for qp in range(NPC):
                    ht_sb = ppool.tile([128, PCS], f32, name=f"ht_sb{qp}",
                                       tag="htsb", bufs=2)
                    for half in range(2):
                        qc = 2 * qp + half
                        ht_ps, hslice = bh_group(qc, list(range(8, NKT)),
                                                 emit_out=True)
                        nc.vector.tensor_tensor(
                            out=ht_sb[:, half * QCS:(half + 1) * QCS],
                            in0=ht_ps[:], in1=hslice, op=ALU.add)
                    # Scalar ring is free by now (all exps done).
                    nc.scalar.dma_start(
                        out=out_d[:, qp * PCS:(qp + 1) * PCS], in_=ht_sb[:])

    nc.compile()
    return nc


def _get_nc():
    if "nc" not in _CACHE:
        _CACHE["nc"] = _build()
    return _CACHE["nc"]


def _blk(xt):
    """[1024, n*512] transposed activations -> [n, 128, 8, 512] blocked."""
    n = xt.shape[1] // QCS
    return np.ascontiguousarray(
        xt.reshape(NDT, 128, n, QCS).transpose(2, 1, 0, 3))


def _make_in_maps(inp_q, inp_k, inp_v, Wq, bq, Wk, bk, Wv, bv):
    bf = ml_dtypes.bfloat16

    def wt(W):  # [128, 1024] -> W.T tiled [p, dt, o] (SBUF layout)
        return W.T.reshape(NDT, 128, DH).transpose(1, 0, 2)

    cq = np.zeros((128, 10, 128), np.float32)
    cq[:, 0:NDT, :] = wt(Wq)
    cq[:, 8, 0] = bq
    cq[:, 8, 1] = bk
    cq[:, 9, :] = bv[None, :]
    cq_np = np.ascontiguousarray(cq).astype(bf)

    ck = np.zeros((128, 16, 128), np.float32)
    ck[:, 0:NDT, :] = wt(Wk)
    ck[:, NDT:2 * NDT, :] = wt(Wv)
    ck_np = np.ascontiguousarray(ck).astype(bf)

    in_maps = []
    for b in range(B):
        xq_np = _blk(inp_q[b].T).astype(bf)
        for h in range(2):
            sl = slice(h * KCH, (h + 1) * KCH)
            xk_np = _blk(inp_k[b, sl].T).astype(bf)
            xv_np = _blk(inp_v[b, sl].T).astype(bf)
            in_maps.append({
                "xq_t": xq_np, "xk_t": xk_np, "xv_t": xv_np,
                "consts_q": cq_np, "consts_kv": ck_np,
            })
    return in_maps


def kernel(inp_q, inp_k, inp_v, Wq, bq, Wk, bk, Wv, bv, _trace=False):
    from concourse.bass_utils import run_bass_kernel_spmd

    inp_q = np.asarray(inp_q, np.float32)
    inp_k = np.asarray(inp_k, np.float32)
    inp_v = np.asarray(inp_v, np.float32)
    Wq, bq = np.asarray(Wq, np.float32), np.asarray(bq, np.float32)
    Wk, bk = np.asarray(Wk, np.float32), np.asarray(bk, np.float32)
    Wv, bv = np.asarray(Wv, np.float32), np.asarray(bv, np.float32)

    nc = _get_nc()
    in_maps = _make_in_maps(inp_q, inp_k, inp_v, Wq, bq, Wk, bk, Wv, bv)
    res = run_bass_kernel_spmd(nc, in_maps, core_ids=list(range(NCORES)),
                               trace=_trace)
    if _trace:
        _CACHE["last_result"] = res

    H = np.empty((B, L, DH), np.float32)
    for b in range(B):
        H[b] = (res.results[2 * b]["out"] + res.results[2 * b + 1]["out"]).T
    return H


# revision 4
# speedup vs baseline: 1.2164x; 1.2164x over previous
"""Distributed AttentionLayer kernel for one TRN2 chip (8 NeuronCores).

Reference computation (note the unusual softmax over the QUERY axis):
    Q = Xq @ Wq.T + bq                      [B, L, 128]
    K = Xk @ Wk.T + bk
    V = Xv @ Wv.T + bv
    S = softmax(Q @ K.T / sqrt(128), axis=q)    (normalized over queries)
    H = S @ V                               [B, L, 128]

Sharding: 8 cores = 4 batches x 2 key-chunks; exact partial-H per core
(zero collectives), host adds the two k-chunk partials per batch.

All-bf16 value path (fp8 rejected: per-element quantization noise of any
operand in the E/V/H chain lands on H at full strength -- for iid data
noise and signal both grow as sqrt(N), nothing washes out -- and fp8's
~3% rms blows the 2e-2 gate).

v4: the schedule is rebuilt around the measured DMA arrival train.  The
input stream (17.4MB on one HWDGE queue) runs at a steady ~415GB/s and
is the startup pacer; v3 wasted 12us by (a) shipping xv0 before
xq0/xk0, (b) wave0 needing BOTH xk0 and xk1, (c) 36 V matmuls parked in
the PE's in-order FIFO ahead of the first score matmuls.  v4 ships
tensors in exact first-use order, opens with a minimal 3.9MB prefix
(consts_q + xq0 + xq1 + consts_kv + xk0 -> first exp ~18us vs 30.3us),
and weaves the V projections into the sweep where xv arrives.  ET/hsum
SBUF slots are paired with input tiles that are provably dead >=1.2us
before first write.  PE warmup matmuls (full 128-partition stationary)
open the HAM clock gate during the DMA lead-in.

Engine budget per core: ACT 64 exps ~72us + 16 colsum accums (the
qp0 partials ride the exp ACTIVATE's accumulator); DVE carries qp1-3
colsum reduces + evacuations (~75us); GPSIMD takes the Vs scaling;
PE ~107us busy incl. overheads.
"""

import math

import numpy as np
import ml_dtypes

B, L, DM, DH = 4, 4096, 1024, 128
NCORES = 8
KCH = L // 2            # 2048 keys per core
QCS = 512               # matmul moving-dim chunk (one PSUM bank of f32)
NQC = L // QCS          # 8
PCS = 1024              # 2-bank PSUM tile width (pairs of QCS chunks)
NPC = L // PCS          # 4 query pair-chunks
NKT = KCH // 128        # 16 key tiles per core
NDT = DM // 128         # 8 d_model tiles
NKC = KCH // QCS        # 4 key 512-chunks for the K/V loads
SCALE = 1.0 / math.sqrt(DH)

_CACHE = {}


def _build():
    import concourse.tile as tile
    from concourse import bacc, mybir

    f32 = mybir.dt.float32
    bf16 = mybir.dt.bfloat16
    AX = mybir.AxisListType
    ALU = mybir.AluOpType
    ACT = mybir.ActivationFunctionType

    nc = bacc.Bacc("TRN2", target_bir_lowering=False, debug=False,
                   num_devices=NCORES)

    # Host-side layouts:
    #   x*_t: [blk, p, dt, c]  with d = dt*128+p and l = blk*512+c
    #   consts_q:  [p, 10, 128] bf16: rows 0-7 wq (W.T tiled [p, dt, o]),
    #              row 8 biases (col0 bq, col1 bk), row 9 bv replicated
    #   consts_kv: [p, 16, 128] bf16: rows 0-7 wk, rows 8-15 wv
    xq_d = nc.dram_tensor("xq_t", [NQC, 128, NDT, QCS], bf16, kind="ExternalInput")
    xk_d = nc.dram_tensor("xk_t", [NKC, 128, NDT, QCS], bf16, kind="ExternalInput")
    xv_d = nc.dram_tensor("xv_t", [NKC, 128, NDT, QCS], bf16, kind="ExternalInput")
    cq_d = nc.dram_tensor("consts_q", [128, 10, 128], bf16, kind="ExternalInput")
    ck_d = nc.dram_tensor("consts_kv", [128, 16, 128], bf16, kind="ExternalInput")
    out_d = nc.dram_tensor("out", [DH, L], f32, kind="ExternalOutput")

    with tile.TileContext(nc) as tc:
        with tc.tile_pool(name="const", bufs=1) as cpool, \
             tc.tile_pool(name="persist", bufs=1) as ppool, \
             tc.tile_pool(name="psmm", bufs=3, space="PSUM") as psmm, \
             tc.tile_pool(name="psaux", bufs=2, space="PSUM") as psaux:

            # ---------- constants ----------
            cq_sb = cpool.tile([128, 10, 128], bf16, name="cq_sb", tag="cq")
            ck_sb = cpool.tile([128, 16, 128], bf16, name="ck_sb", tag="ck")
            warm_st = cpool.tile([128, DH], bf16, name="warm_st", tag="wst")
            warm_mv = cpool.tile([128, QCS], bf16, name="warm_mv", tag="wmv")
            ones_sb = cpool.tile([1, DH], bf16, name="ones_sb", tag="ones")

            wq_sb = cq_sb[:, 0:NDT, :]
            bv_sb = cq_sb[0:1, 9, :]
            wk_sb = ck_sb[:, 0:NDT, :]
            wv_sb = ck_sb[:, NDT:2 * NDT, :]

            # consts_q leads the single sync HWDGE queue; everything ships
            # in exact first-use order (single queue = in-order delivery).
            nc.sync.dma_start(out=cq_sb[:], in_=cq_d[:])
            nc.vector.memset(warm_st[:], 0.125)
            nc.vector.memset(warm_mv[:], 0.125)
            nc.vector.memset(ones_sb[:], 1.0)

            # Per-v_group gated copies of bv: bv2[g] = 0*et_gate + bv.  The
            # artificial data dep on a mid-stream ET slice stops the legacy
            # CoreSim scheduler (whose DMA model ignores queue serialization)
            # from hoisting the V matmuls to the front of the PE FIFO where
            # they head-of-line block on xv0 for ~25us (measured in v4;
            # bass_priority is ignored by this scheduler).
            bv2_sb = cpool.tile([1, 4, DH], bf16, name="bv2_sb", tag="bv2")

            # tensor_scalar_add wants f32 scalars; up-convert the bf16
            # biases once (tiny DVE copy).
            bqk_sb = cpool.tile([128, 2], f32, name="bqk_sb", tag="bqk")
            nc.vector.tensor_copy(out=bqk_sb[:], in_=cq_sb[:, 8, 0:2])
            bq_sb = bqk_sb[:, 0:1]
            bk_sb = bqk_sb[:, 1:2]

            # ---------- persistent activations ----------
            qt_sb = ppool.tile([128, L], bf16, name="qt_sb", tag="qt")      # Q^T [o, q]
            kt_sb = ppool.tile([128, KCH], bf16, name="kt_sb", tag="kt")    # K^T [o, k]
            v_sb = ppool.tile([128, NKT, DH], bf16, name="v_sb", tag="v")   # V  [k, kt, v]
            vs_sb = ppool.tile([128, NKT, DH], bf16, name="vs_sb", tag="vs")
            cs_parts = ppool.tile([128, NKT, NPC], f32, name="cs_parts", tag="csp")
            cs_sum = ppool.tile([128, NKT], f32, name="cs_sum", tag="css")
            cs_rec = ppool.tile([128, NKT], f32, name="cs_rec", tag="csr")

            # X, ET and hsum tiles share one 20-slot 8KB rotation; the
            # allocation order pairs each late tile with an input tile
            # that dies >=1.2us before the late tile's first write
            # (arrival/death schedule in the header comment).
            with tc.tile_pool(name="xe", bufs=20) as xpool:
                xq_sbs = [None] * NQC
                xk_sbs = [None] * NKC
                xv_sbs = [None] * NKC
                alloc_seq = [("q", 0), ("q", 1), ("k", 0), ("k", 1),
                             ("q", 2), ("q", 3), ("k", 2),
                             ("q", 4), ("q", 5), ("q", 6), ("q", 7),
                             ("k", 3),
                             ("v", 0), ("v", 1), ("v", 2), ("v", 3)]
                for kind, j in alloc_seq:
                    t = xpool.tile([128, NDT, QCS], bf16,
                                   name=f"x{kind}_sb{j}", tag="xe")
                    if kind == "q":
                        xq_sbs[j] = t
                    elif kind == "k":
                        xk_sbs[j] = t
                    else:
                        xv_sbs[j] = t
                et_ts = [xpool.tile([128, L], bf16, name=f"et_sb{kt}",
                                    tag="xe") for kt in range(NKT)]
                hsum_ts = [xpool.tile([128, L // 2], f32, name=f"hsum{i}",
                                      tag="xe") for i in range(2)]

                # DMA ship order == first-use order.
                nc.sync.dma_start(out=xq_sbs[0][:], in_=xq_d[0])
                nc.sync.dma_start(out=xq_sbs[1][:], in_=xq_d[1])
                nc.sync.dma_start(out=ck_sb[:], in_=ck_d[:])
                for kind, j in [("k", 0), ("k", 1), ("q", 2), ("q", 3),
                                ("k", 2), ("q", 4), ("q", 5), ("q", 6),
                                ("q", 7), ("k", 3),
                                ("v", 0), ("v", 1), ("v", 2), ("v", 3)]:
                    if kind == "q":
                        nc.sync.dma_start(out=xq_sbs[j][:], in_=xq_d[j])
                    elif kind == "k":
                        nc.sync.dma_start(out=xk_sbs[j][:], in_=xk_d[j])
                    else:
                        nc.sync.dma_start(out=xv_sbs[j][:], in_=xv_d[j])

                # ---------- PE warmup: open the HAM clock gate ----------
                # Full 128-partition matmuls on memset data (1-partition
                # rank-1 warmups in v3 did NOT trip the activity monitor).
                warm_ps = psaux.tile([128, QCS], f32, name="warm_ps",
                                     tag="aux")
                for i in range(12):
                    nc.tensor.matmul(out=warm_ps[:], lhsT=warm_st[:],
                                     rhs=warm_mv[:], start=True, stop=True)

                def qt_proj(qp):
                    qt_ps = psmm.tile([128, PCS], f32, name=f"qt_ps{qp}", tag="mm")
                    for half in range(2):
                        qc = 2 * qp + half
                        for dt in range(NDT):
                            nc.tensor.matmul(
                                out=qt_ps[:, half * QCS:(half + 1) * QCS],
                                lhsT=wq_sb[:, dt, :],
                                rhs=xq_sbs[qc][:, dt, :],
                                start=(dt == 0), stop=(dt == NDT - 1))
                    nc.vector.tensor_scalar_add(
                        out=qt_sb[:, qp * PCS:(qp + 1) * PCS], in0=qt_ps[:],
                        scalar1=bq_sb)

                def kt_proj(kc):
                    kt_ps = psmm.tile([128, QCS], f32, name=f"kt_ps{kc}", tag="mm")
                    for dt in range(NDT):
                        nc.tensor.matmul(
                            out=kt_ps[:],
                            lhsT=wk_sb[:, dt, :],
                            rhs=xk_sbs[kc][:, dt, :],
                            start=(dt == 0), stop=(dt == NDT - 1))
                    nc.vector.tensor_scalar_add(
                        out=kt_sb[:, kc * QCS:(kc + 1) * QCS], in0=kt_ps[:],
                        scalar1=bk_sb)

                def st_cell(kt, qp):
                    """Scores + exp for one (kt, qp) cell; colsum partial on
                    the ACT accumulator for qp0, DVE reduce for qp1-3."""
                    st_ps = psmm.tile([128, PCS], f32,
                                      name=f"st_ps_{kt}_{qp}", tag="mm")
                    for half in range(2):
                        qc = 2 * qp + half
                        nc.tensor.matmul(
                            out=st_ps[:, half * QCS:(half + 1) * QCS],
                            lhsT=kt_sb[:, kt * 128:(kt + 1) * 128],
                            rhs=qt_sb[:, qc * QCS:(qc + 1) * QCS],
                            start=True, stop=True)
                    et_slice = et_ts[kt][:, qp * PCS:(qp + 1) * PCS]
                    if qp == 0:
                        nc.scalar.activation(
                            out=et_slice, in_=st_ps[:], func=ACT.Exp,
                            scale=SCALE, accum_out=cs_parts[:, kt, qp:qp + 1])
                    else:
                        nc.scalar.activation(
                            out=et_slice, in_=st_ps[:], func=ACT.Exp,
                            scale=SCALE)
                        nc.vector.tensor_reduce(
                            out=cs_parts[:, kt, qp:qp + 1], in_=et_slice,
                            axis=AX.X, op=ALU.add)

                def finish_kt(kt):
                    """Total colsum -> reciprocal -> scaled V for one k-tile."""
                    nc.vector.tensor_reduce(
                        out=cs_sum[:, kt:kt + 1], in_=cs_parts[:, kt, :],
                        axis=AX.X, op=ALU.add)
                    nc.vector.reciprocal(out=cs_rec[:, kt:kt + 1],
                                         in_=cs_sum[:, kt:kt + 1])
                    nc.vector.tensor_scalar_mul(
                        out=vs_sb[:, kt, :], in0=v_sb[:, kt, :],
                        scalar1=cs_rec[:, kt:kt + 1])

                def v_group(g, gate):
                    """V[k,v] for k-tiles 4g..4g+3 (one xv block): bias
                    rank-1 + 8 d-tiles each; one PSUM bank, one evac.  The
                    bias matmul reads bv2[g] whose producer depends on
                    `gate`, anchoring the whole PSUM accumulation group at
                    the right spot in the PE FIFO."""
                    nc.vector.scalar_tensor_tensor(
                        out=bv2_sb[0:1, g, :], in0=gate, scalar=0.0,
                        in1=bv_sb, op0=ALU.mult, op1=ALU.add)
                    v_ps = psaux.tile([128, 4, DH], f32, name=f"v_ps{g}",
                                      tag="aux")
                    for i in range(4):
                        nc.tensor.matmul(out=v_ps[:, i, :], lhsT=ones_sb[:],
                                         rhs=bv2_sb[0:1, g, :],
                                         start=True, stop=False)
                        for dt in range(NDT):
                            nc.tensor.matmul(
                                out=v_ps[:, i, :],
                                lhsT=xv_sbs[g][:, dt, i * 128:(i + 1) * 128],
                                rhs=wv_sb[:, dt, :],
                                start=False, stop=(dt == NDT - 1))
                    nc.vector.tensor_copy(out=v_sb[:, 4 * g:4 * g + 4, :],
                                          in_=v_ps[:])

                def bh_group(qc, kts, emit_out):
                    """H^T partial over `kts` for one q-column; first half
                    parks in hsum (f32 SBUF), second half adds it back."""
                    ht_ps = psaux.tile([128, QCS], f32,
                                       name=f"ht_ps_{qc}_{kts[0]}", tag="aux")
                    for i, kt in enumerate(kts):
                        nc.tensor.matmul(
                            out=ht_ps[:],
                            lhsT=vs_sb[:, kt, :],
                            rhs=et_ts[kt][:, qc * QCS:(qc + 1) * QCS],
                            start=(i == 0), stop=(i == len(kts) - 1))
                    hslice = hsum_ts[qc // 4][:, (qc % 4) * QCS:
                                              (qc % 4 + 1) * QCS]
                    if not emit_out:
                        nc.vector.tensor_copy(out=hslice, in_=ht_ps[:])
                        return None
                    return ht_ps, hslice

                # ---------- schedule: exact DMA-arrival order ----------
                qt_proj(0)
                kt_proj(0)
                for kt in range(0, 4):
                    st_cell(kt, 0)
                kt_proj(1)
                for kt in range(4, 8):
                    st_cell(kt, 0)
                qt_proj(1)
                for kt in range(0, 8):
                    st_cell(kt, 1)
                kt_proj(2)
                st_cell(8, 0)
                st_cell(9, 0)
                qt_proj(2)
                st_cell(10, 0)
                st_cell(11, 0)
                qt_proj(3)
                kt_proj(3)
                for kt in range(12, 16):
                    st_cell(kt, 0)
                for kt in range(8, 16):
                    st_cell(kt, 1)

                # kt-major sweep: each k-tile's colsum closes right after
                # its (qp2, qp3) cells; V projections woven where xv lands;
                # kt0-7 H partials woven from kt>=8.
                # Gates: ET slices produced around each xv block's real
                # arrival time (st(12,0) ~46us, st(9,1) ~48us, ...).
                VG_AT = {0: (0, et_ts[12][0:1, 0:DH]),
                         1: (1, et_ts[9][0:1, PCS:PCS + DH]),
                         3: (2, et_ts[11][0:1, PCS:PCS + DH]),
                         5: (3, et_ts[13][0:1, PCS:PCS + DH])}
                for kt in range(NKT):
                    st_cell(kt, 2)
                    if kt in VG_AT:
                        g, gate = VG_AT[kt]
                        v_group(g, gate)
                    st_cell(kt, 3)
                    finish_kt(kt)
                    if kt >= 8:
                        bh_group(kt - 8, list(range(8)), emit_out=False)

                # ---------- H^T second half (kt8-15) + combine ----------
                for qp in range(NPC):
                    ht_sb = ppool.tile([128, PCS], f32, name=f"ht_sb{qp}",
                                       tag="htsb", bufs=2)
                    for half in range(2):
                        qc = 2 * qp + half
                        ht_ps, hslice = bh_group(qc, list(range(8, NKT)),
                                                 emit_out=True)
                        nc.vector.tensor_tensor(
                            out=ht_sb[:, half * QCS:(half + 1) * QCS],
                            in0=ht_ps[:], in1=hslice, op=ALU.add)
                    # Scalar ring is free by now (all exps done).
                    nc.scalar.dma_start(
                        out=out_d[:, qp * PCS:(qp + 1) * PCS], in_=ht_sb[:])

    nc.compile()
    return nc


def _get_nc():
    if "nc" not in _CACHE:
        _CACHE["nc"] = _build()
    return _CACHE["nc"]


def _blk(xt):
    """[1024, n*512] transposed activations -> [n, 128, 8, 512] blocked."""
    n = xt.shape[1] // QCS
    return np.ascontiguousarray(
        xt.reshape(NDT, 128, n, QCS).transpose(2, 1, 0, 3))


def _make_in_maps(inp_q, inp_k, inp_v, Wq, bq, Wk, bk, Wv, bv):
    bf = ml_dtypes.bfloat16

    def wt(W):  # [128, 1024] -> W.T tiled [p, dt, o] (SBUF layout)
        return W.T.reshape(NDT, 128, DH).transpose(1, 0, 2)

    cq = np.zeros((128, 10, 128), np.float32)
    cq[:, 0:NDT, :] = wt(Wq)
    cq[:, 8, 0] = bq
    cq[:, 8, 1] = bk
    cq[:, 9, :] = bv[None, :]
    cq_np = np.ascontiguousarray(cq).astype(bf)

    ck = np.zeros((128, 16, 128), np.float32)
    ck[:, 0:NDT, :] = wt(Wk)
    ck[:, NDT:2 * NDT, :] = wt(Wv)
    ck_np = np.ascontiguousarray(ck).astype(bf)

    in_maps = []
    for b in range(B):
        xq_np = _blk(inp_q[b].T).astype(bf)
        for h in range(2):
            sl = slice(h * KCH, (h + 1) * KCH)
            xk_np = _blk(inp_k[b, sl].T).astype(bf)
            xv_np = _blk(inp_v[b, sl].T).astype(bf)
            in_maps.append({
                "xq_t": xq_np, "xk_t": xk_np, "xv_t": xv_np,
                "consts_q": cq_np, "consts_kv": ck_np,
            })
    return in_maps


def kernel(inp_q, inp_k, inp_v, Wq, bq, Wk, bk, Wv, bv, _trace=False):
    from concourse.bass_utils import run_bass_kernel_spmd

    inp_q = np.asarray(inp_q, np.float32)
    inp_k = np.asarray(inp_k, np.float32)
    inp_v = np.asarray(inp_v, np.float32)
    Wq, bq = np.asarray(Wq, np.float32), np.asarray(bq, np.float32)
    Wk, bk = np.asarray(Wk, np.float32), np.asarray(bk, np.float32)
    Wv, bv = np.asarray(Wv, np.float32), np.asarray(bv, np.float32)

    nc = _get_nc()
    in_maps = _make_in_maps(inp_q, inp_k, inp_v, Wq, bq, Wk, bk, Wv, bv)
    res = run_bass_kernel_spmd(nc, in_maps, core_ids=list(range(NCORES)),
                               trace=_trace)
    if _trace:
        _CACHE["last_result"] = res

    H = np.empty((B, L, DH), np.float32)
    for b in range(B):
        H[b] = (res.results[2 * b]["out"] + res.results[2 * b + 1]["out"]).T
    return H
